# revision 26
# baseline (speedup 1.0000x reference)
"""Trainium2 Bass kernel for nn_ContrastSSIMLoss.

loss = mean_{b,h,w,s} | C_o(s,h,w) - C_s(s,h,w) |  over 120 shifts s=(i,j),
where C_img(s,h,w) = sum_c |img[c,h+5,w+5] - img[c,h+5+i,w+5+j]|,
output domain 246x246, B=16, C=3, H=256, w=5.

Strategy
- Pure data parallel: batch dim sharded 2-per-core across 8 NeuronCores.
- Half-shift trick: for s=(i,j) with i>0 or (i==0 and j>0), the map for -s is
  a translated copy of the map for s.  Compute F(y,x) = |A_o - A_s| once per
  half-shift on an extended domain, then two window sums:
    W1 = sum over y,x in [0,246)^2                   (contribution of s)
    W2 = sum over y in [-i,246-i), x in [-j,246-j)   (contribution of -s)
- Partition layout p = 2g + b (g in 0..62, b in 0..1): partition owns image
  rows [4g, 4g+9) of batch b.  Host pre-builds the halo strips (zero-padded)
  so interior partitions (g in [2,61), i.e. p in [4,122)) are valid for BOTH
  windows of every shift.
- Engine split per half-shift:
    DVE   : image-0 sub (2x bf16), image-0 abs as bitwise-AND 0x7fff
            (tensor_scalar 4x mode, flat planes), both channel-sum adds.
    PE    : image-1 sub as paired matmuls (+I on center, -I on shifted,
            accumulated in PSUM; 12 bank-sized matmuls per shift).
    ACT   : image-1 abs straight out of PSUM (2 instrs), plus ONE merged
            dual-window interior accum: activation(Abs, accum_out) over
            [2 windows, 4 rows, 246 cols]; interior slots need no
            per-window split since only their total enters the loss.
    DVE   : also the cross-image f-sub - it must NOT run on gpsimd, whose
            streaming steals the second DVE SBUF port and throttles every
            2-port DVE instruction (~35% DVE slowdown measured).
    SYNC  : boundary stash DMAs (4 per shift) - rows of the 8 boundary
            partitions are spread across all 128 partitions of a stash
            tile; staged DVE reduces yield per-row sums; host masks.
- Host computes the final mean in f64 from interior slots + boundary sums.
- kernel() runs the NEFF twice and reports the warm second execution
  (the first execution after model load is ~70us slower).
"""

import numpy as np

W = 5
H = 256
OUT = H - 2 * W          # 246
B_TOTAL, C = 16, 3
NCORES = 8
NB = B_TOTAL // NCORES   # 2 batches per core
RPB = 4                  # owned rows per block
SROWS = RPB + W          # 9 strip rows per partition
PADL = 8                 # left pad cols in strip (even => 4B-aligned bf16)
SCOLS = PADL + H + 8     # 272 padded strip row length
HS = [(i, j) for i in range(0, W + 1) for j in range(-W, W + 1)
      if i > 0 or (i == 0 and j > 0)]
HS.sort(key=lambda s: (abs(s[1]) % 2, s[0], s[1]))  # even-j first
assert len(HS) == 60
NSHIFT = len(HS)
NSTASH = NSHIFT // 2           # stash slots per partition (2 shifts / 128p)
# stash row layout: p' = 64*(k%2) + 16*(2*win + rng) + idx ; slot s = k//2
# rng 0 -> src partitions [0,4), rng 1 -> src partitions [122,126)
# idx = 4*(local partition) + row

_COMPILED = None
LAST_RESULTS = None


def _build():
    import concourse.bass as bass
    import concourse.mybir as mybir
    from concourse import bacc, tile

    f32 = mybir.dt.float32
    bf16 = mybir.dt.bfloat16
    u16 = mybir.dt.uint16
    SUB = mybir.AluOpType.subtract
    ADD = mybir.AluOpType.add
    AND = mybir.AluOpType.bitwise_and
    ABS = mybir.ActivationFunctionType.Abs
    AX = mybir.AxisListType.X

    nc = bacc.Bacc("TRN2", target_bir_lowering=False, debug=False,
                   num_devices=NCORES)

    # host-prepped strips: [128, im, C, SROWS, H], p = 2g+b (126/127 zero)
    strips_dram = nc.dram_tensor("strips", [128, 2, C, SROWS, H], bf16,
                                 kind="ExternalInput")
    # [I128 | -I128] as bf16
    ident_dram = nc.dram_tensor("ident", [128, 256], bf16,
                                kind="ExternalInput")
    islots_dram = nc.dram_tensor("islots", [128, NSHIFT], f32,
                                 kind="ExternalOutput")
    bred_dram = nc.dram_tensor("bred", [128, NSTASH], f32,
                               kind="ExternalOutput")

    with tile.TileContext(nc) as tc:
        with (
            tc.tile_pool(name="strips", bufs=1) as spool,
            tc.tile_pool(name="dw", bufs=3) as dpool,
            tc.tile_pool(name="aw", bufs=3) as adpool,
            tc.tile_pool(name="amaps", bufs=3) as apool,
            tc.tile_pool(name="fmaps", bufs=4) as fpool,
            tc.tile_pool(name="res", bufs=1) as rpool,
            tc.tile_pool(name="psum", bufs=1, space="PSUM") as ppool,
        ):
            sE = spool.tile([128, 2, C, SROWS, SCOLS], bf16, name="sE")
            # odd-parity copy needed only for the DVE image-0 path
            sO = spool.tile([128, C, SROWS, SCOLS], bf16, name="sO")
            ident = spool.tile([128, 256], bf16, name="ident")
            islots = rpool.tile([128, NSHIFT], f32, name="islots")
            stash = rpool.tile([128, NSTASH, OUT], bf16, name="stash")
            bred = rpool.tile([128, NSTASH], f32, name="bred")
            scr = rpool.tile([128, 2, RPB, OUT], bf16, name="scr")

            hsrc = (strips_dram.tensor if hasattr(strips_dram, "tensor")
                    else strips_dram)
            qi = 0
            for im in range(2):
                for cc in range(C):
                    for ph in range(2):
                        coff = ((ph * 64) * 2 * C + im * C + cc) * SROWS * H
                        eng = nc.gpsimd if qi % 2 == 0 else nc.sync
                        qi += 1
                        eng.dma_start(
                            out=sE[ph * 64:ph * 64 + 64, im, cc, :,
                                   PADL:PADL + H],
                            in_=bass.AP(hsrc, coff,
                                        [[2 * C * SROWS * H, 64],
                                         [H, SROWS], [1, H]]))
            hid = (ident_dram.tensor if hasattr(ident_dram, "tensor")
                   else ident_dram)
            nc.gpsimd.dma_start(out=ident[:],
                                in_=bass.AP(hid, 0, [[256, 128], [1, 256]]))

            def emit_sub(k):
                i, j = HS[k]
                # ---- image 0 on DVE: sub + bitwise abs (4x)
                d0 = dpool.tile([128, C, RPB, H], bf16, tag="d0",
                                name=f"d0_{k}")
                center = sE[:, 0, :, 0:RPB, PADL:PADL + H]
                if j % 2 == 0:
                    shifted = sE[:, 0, :, i:i + RPB, PADL + j:PADL + j + H]
                else:
                    shifted = sO[:, :, i:i + RPB,
                                 PADL + 1 + j:PADL + 1 + j + H]
                nc.vector.tensor_tensor(out=d0[:], in0=center, in1=shifted,
                                        op=SUB)
                ad = adpool.tile([128, 2, C, RPB, H], bf16, tag="ad",
                                 name=f"ad{k}")
                nc.vector.tensor_scalar(
                    out=ad.bitcast(u16)[:, 0].rearrange("p c r x -> p (c r x)"),
                    in0=d0.bitcast(u16).rearrange("p c r x -> p (c r x)"),
                    scalar1=0x7FFF, scalar2=None, op0=AND)
                # ---- image 1 on PE: d1 = center - shifted into PSUM
                # 6 bank chunks: (cc, rh) -> rows [2rh, 2rh+2)
                for half in range(2):
                    pt = ppool.tile([128, 3 * 512], f32, tag=f"ps{half}",
                                    name=f"ps{half}_{k}")
                    chunks = [(cc, rh) for cc in range(C) for rh in range(2)
                              ][half * 3:half * 3 + 3]
                    for q, (cc, rh) in enumerate(chunks):
                        nc.tensor.matmul(
                            pt[:, q * 512:(q + 1) * 512],
                            ident[:, 0:128],
                            sE[:, 1, cc, 2 * rh:2 * rh + 2, PADL:PADL + H],
                            start=True, stop=False)
                    for q, (cc, rh) in enumerate(chunks):
                        nc.tensor.matmul(
                            pt[:, q * 512:(q + 1) * 512],
                            ident[:, 128:256],
                            sE[:, 1, cc, 2 * rh + i:2 * rh + 2 + i,
                               PADL + j:PADL + j + H],
                            start=False, stop=True)
                    # ACT: abs PSUM -> ad image-1 flat half
                    nc.scalar.activation(
                        out=ad[:, 1].rearrange("p c r x -> p (c r x)")
                        [:, half * 1536:half * 1536 + 1536],
                        in_=pt[:], func=ABS)
                return ad

            def emit_adds(k, ad):
                # lagged one stage behind emit_sub so the in-order DVE queue
                # never waits on ACT's same-shift PSUM abs
                i, j = HS[k]
                xlo = min(W, W - j) & ~1
                xhi = max(W + OUT, W + OUT - j)
                xhi += xhi & 1
                a01 = apool.tile([128, 2, RPB, H], bf16, tag="a01",
                                 name=f"a01{k}")
                nc.vector.tensor_tensor(out=a01[:, :, :, xlo:xhi],
                                        in0=ad[:, :, 0, :, xlo:xhi],
                                        in1=ad[:, :, 1, :, xlo:xhi], op=ADD)
                a = apool.tile([128, 2, RPB, H], bf16, tag="a", name=f"a{k}")
                nc.vector.tensor_tensor(out=a[:, :, :, xlo:xhi],
                                        in0=a01[:, :, :, xlo:xhi],
                                        in1=ad[:, :, 2, :, xlo:xhi], op=ADD)
                if k == 6:
                    # odd-parity copy (image 0 only; first odd-j shift k=30)
                    nc.scalar.copy(out=sO[:, :, :, PADL + 1:PADL + 1 + H],
                                   in_=sE[:, 0, :, :, PADL:PADL + H])
                return a

            def emit_fsub(k, a):
                # on DVE: gpsimd streaming would steal the second DVE SBUF
                # port and throttle every 2-port DVE instruction
                i, j = HS[k]
                xlo = min(W, W - j) & ~1
                xhi = max(W + OUT, W + OUT - j)
                xhi += xhi & 1
                f = fpool.tile([128, RPB, H], bf16, tag="f", name=f"f{k}")
                nc.vector.tensor_tensor(out=f[:, :, xlo:xhi],
                                        in0=a[:, 0, :, xlo:xhi],
                                        in1=a[:, 1, :, xlo:xhi], op=SUB)
                return f

            def emit_reds(k, f):
                i, j = HS[k]
                half = 64 * (k % 2)
                s = k // 2
                # merged dual-window interior accum (host uses p in [4,122)):
                # windows start at cols 5 and 5-j; only their SUM matters
                fwin = bass.AP(f[:].tensor, W - max(j, 0),
                               [[RPB * H, 128], [abs(j), 2], [H, RPB],
                                [1, OUT]])
                nc.scalar.activation(
                    out=scr[:], in_=fwin, func=ABS,
                    accum_out=islots[:, k:k + 1])
                # boundary stash: 2 DMAs per window
                for win, x0 in ((0, W), (1, W - j)):
                    for rng, plo in ((0, 0), (1, 122)):
                        dst0 = half + 16 * (2 * win + rng)
                        nc.sync.dma_start(
                            out=stash[dst0:dst0 + 16, s, 0:OUT],
                            in_=f[plo:plo + 4, :, x0:x0 + OUT])

            st_a = {}
            st_f = {}
            for k in range(NSHIFT + 2):
                if k < NSHIFT:
                    st_a[k] = emit_adds(k, emit_sub(k))
                if 0 <= k - 1 < NSHIFT:
                    st_f[k - 1] = emit_fsub(k - 1, st_a.pop(k - 1))
                if 0 <= k - 2 < NSHIFT:
                    emit_reds(k - 2, st_f.pop(k - 2))
                # reduce finished stash pieces off the critical tail;
                # the last piece is kept small (3 slots) to shorten the
                # end-of-kernel serial chain
                for qlo, qhi in ((0, 7), (7, 14), (14, 21), (21, 27)):
                    if k == 2 * qhi + 1:
                        nc.vector.tensor_reduce(
                            out=bred[:, qlo:qhi], in_=stash[:, qlo:qhi],
                            axis=AX, op=ADD, apply_absolute_value=True)

            # boundary per-row sums for the last 3 stash slots
            nc.vector.tensor_reduce(out=bred[:, 27:],
                                    in_=stash[:, 27:], axis=AX,
                                    op=ADD, apply_absolute_value=True)
            nc.sync.dma_start(out=islots_dram[:], in_=islots[:])
            nc.sync.dma_start(out=bred_dram[:], in_=bred[:])

    nc.compile()
    return nc


def _prep_strips(orig, simu):
    """[NB,C,H,H] x2 (bf16) -> [128, 2, C, SROWS, H] with p = 2g+b."""
    import ml_dtypes
    out = np.zeros((64, NB, 2, C, SROWS, H), dtype=ml_dtypes.bfloat16)
    imgs = np.stack([orig, simu], axis=1)  # [NB, 2, C, H, H]
    for g in range(63):
        r0 = 4 * g
        r1 = min(H, r0 + SROWS)
        out[g, :, :, :, 0:r1 - r0] = imgs[:, :, :, r0:r1]
    return np.ascontiguousarray(out.reshape(128, 2, C, SROWS, H))


def _masks():
    """Valid-row mask over the boundary stash layout."""
    bmask = np.zeros((128, NSTASH), dtype=bool)
    for k, (i, j) in enumerate(HS):
        half = 64 * (k % 2)
        s = k // 2
        for win in range(2):
            ylo, yhi = (0, OUT) if win == 0 else (-i, OUT - i)
            for rng, plo in ((0, 0), (1, 122)):
                for idx in range(16):
                    p_src = plo + idx // 4
                    r = idx % 4
                    g = p_src // 2
                    y = RPB * g - W + r
                    if ylo <= y < yhi:
                        bmask[half + 16 * (2 * win + rng) + idx, s] = True
    return bmask


def _inject_ntff_hook():
    """Best-effort: register the axon NTFF profile hook so trace=True works."""
    import sys, types
    if "antenv.axon_hooks" in sys.modules:
        return
    try:
        import trn_agent_boot.trn_boot as tb
        hook = tb._ntff_profile_via_ctypes('/opt/axon/libaxon_pjrt.so')
    except Exception:
        return
    mod = types.ModuleType("antenv.axon_hooks")
    _h = [hook]
    mod.set_axon_ntff_profile_hook = lambda h: _h.__setitem__(0, h)
    mod.get_axon_ntff_profile_hook = lambda: _h[0]
    sys.modules["antenv.axon_hooks"] = mod


def kernel(original_image, simulated_image, window_size):
    global _COMPILED, LAST_RESULTS
    assert int(window_size) == W
    import ml_dtypes
    from concourse.bass_utils import run_bass_kernel_spmd

    _inject_ntff_hook()
    if _COMPILED is None:
        _COMPILED = _build()
    nc = _COMPILED

    orig = np.asarray(original_image, dtype=np.float32).astype(
        ml_dtypes.bfloat16)
    simu = np.asarray(simulated_image, dtype=np.float32).astype(
        ml_dtypes.bfloat16)
    eye = np.eye(128, dtype=np.float32)
    ident = np.concatenate([eye, -eye], axis=1).astype(ml_dtypes.bfloat16)
    in_maps = [
        {"strips": _prep_strips(orig[c * NB:(c + 1) * NB],
                                simu[c * NB:(c + 1) * NB]),
         "ident": ident}
        for c in range(NCORES)
    ]
    res = None
    for attempt in range(3):
        try:
            run_bass_kernel_spmd(nc, in_maps, list(range(NCORES)))
            res = run_bass_kernel_spmd(nc, in_maps, list(range(NCORES)))
            break
        except Exception:
            # transient NRT exec-unit failures have been observed on the
            # first execution after load; retry a couple of times
            if attempt == 2:
                raise
            import time
            time.sleep(3)
    LAST_RESULTS = res

    bmask = _masks()
    total = 0.0
    for c in range(NCORES):
        total += res.results[c]["islots"][4:122].sum(dtype=np.float64)
        total += res.results[c]["bred"][bmask].sum(dtype=np.float64)
    loss = total / (B_TOTAL * NSHIFT * 2 * OUT * OUT)
    return np.float32(loss)
